# revision 1
# baseline (speedup 1.0000x reference)
"""GQA kernel for Trainium2 (Bass/Tile), 8 NeuronCores.

Problem: B=2, S=2048, E=1024, NH=16 q-heads, NKV=4 kv-heads (group size 4),
causal mask, fp32 in/out.

Sharding: core = (batch b, kv-group g); each of the 8 cores handles one
batch element and one KV group (4 q heads + their shared kv head). The
out-projection is column-sharded by group; host sums the 4 group partials
per batch.

Layout strategy (everything derived so that NO on-device transposes of big
data-dependent tensors are needed):
  - host supplies x^T (E x S) so projections contract E on partitions
  - Q^T [64d, S] per head and K^T [64d, S] feed the scores matmul directly
  - scores are computed TRANSPOSED: S^T[kj, qi] = (K^T chunk)^T-weights
    against Q^T moving => psum [kj=128, qi=512]; exp on ACT writes A^T
    directly (no max-subtraction: |scaled scores| <= ~3, exp is safe)
  - A^T chunks serve as matmul lhsT for A@V with V in natural [s, d]
    layout; a ones-column appended to V yields the softmax normalizer
  - K=64 score matmuls are packed 2-at-a-time onto PE row-groups 0-63 /
    64-127 (tile_position via base_partition), so the half-depth matmuls
    run concurrently.
"""

import numpy as np
import ml_dtypes

BF16 = ml_dtypes.bfloat16

B, S, E = 2, 2048, 1024
NH, NKV = 16, 4
HD = 64          # head dim
GS = NH // NKV   # 4 q heads per kv head
NHC = 4          # q heads per core
MPC = NHC * HD   # 256 q-out dims per core
P = 128
EC = E // P      # 8 contraction chunks for projections
SC = S // P      # 16 s-chunks of 128
SB = S // 512    # 4 s-blocks of 512
NKJ = S // P     # 16 kj chunks of 128
SCALE = 1.0 / 8.0  # 1/sqrt(HD)

_CACHE = {}


def _build():
    import concourse.bass as bass
    import concourse.tile as tile
    from concourse import bacc, mybir
    from concourse.masks import make_identity

    f32 = mybir.dt.float32
    bf16 = mybir.dt.bfloat16

    nc = bacc.Bacc("TRN2", target_bir_lowering=False, debug=False)

    xq_d = nc.dram_tensor("xqT", [E, S], bf16, kind="ExternalInput").ap()
    xk_d = nc.dram_tensor("xkT", [E, S], bf16, kind="ExternalInput").ap()
    xv_d = nc.dram_tensor("xvT", [E, S], bf16, kind="ExternalInput").ap()
    wq_d = nc.dram_tensor("wqT", [E, MPC], bf16, kind="ExternalInput").ap()
    wk_d = nc.dram_tensor("wkT", [E, HD], bf16, kind="ExternalInput").ap()
    wv_d = nc.dram_tensor("wvT", [E, HD], bf16, kind="ExternalInput").ap()
    wo_d = nc.dram_tensor("woT", [MPC, E], bf16, kind="ExternalInput").ap()
    y_d = nc.dram_tensor("y", [S, E], f32, kind="ExternalOutput").ap()

    xq_r = xq_d.rearrange("(o p) s -> p o s", p=P)
    xk_r = xk_d.rearrange("(o p) s -> p o s", p=P)
    xv_r = xv_d.rearrange("(o p) s -> p o s", p=P)
    y_r = y_d.rearrange("(o p) e -> p o e", p=P)

    with tile.TileContext(nc) as tc:
        with (
            tc.tile_pool(name="const", bufs=1) as const,
            tc.tile_pool(name="xin", bufs=1) as xin,
            tc.tile_pool(name="at", bufs=3) as atpool,
            tc.tile_pool(name="rc", bufs=2) as rcpool,
        ):
            # ---- persistent SBUF tensors
            wq = const.tile([P, EC, MPC], bf16)
            wk = const.tile([P, EC, HD], bf16)
            wv = const.tile([P, EC, HD], bf16)
            wo = const.tile([P, 2, E], bf16)
            ident = const.tile([P, P], bf16)
            masks = const.tile([P, 4, 512], bf16)
            qt2 = const.tile([P, NHC, S], bf16)   # [0:64]=Q^T_h, [64:128]=copy
            kt2 = const.tile([P, S], bf16)        # [0:64]=K^T,  [64:128]=copy
            vaug = const.tile([P, SC, HD + 1], bf16)  # V natural + ones col
            outn = const.tile([P, SC, MPC], bf16)  # normalized attn out [s,m]
            outT = const.tile([P, 2, S], bf16)     # transposed out [m, s]

            nc.sync.dma_start(wq[:], wq_d.rearrange("(o p) m -> p o m", p=P))
            nc.sync.dma_start(wk[:], wk_d.rearrange("(o p) m -> p o m", p=P))
            nc.sync.dma_start(wv[:], wv_d.rearrange("(o p) m -> p o m", p=P))
            nc.sync.dma_start(wo[:], wo_d.rearrange("(o p) e -> p o e", p=P))

            make_identity(nc, ident[:])

            # masks[j][p, fi] = 1.0 if fi - p - 128j >= 0 else 0.0
            nc.gpsimd.memset(masks[:], 1.0)
            for j in range(4):
                nc.gpsimd.affine_select(
                    out=masks[:, j],
                    in_=masks[:, j],
                    compare_op=mybir.AluOpType.is_ge,
                    fill=0.0,
                    base=-128 * j,
                    pattern=[[1, 512]],
                    channel_multiplier=-1,
                )

            xq = xin.tile([P, EC, S], bf16)
            xk = xin.tile([P, EC, S], bf16)
            xv = xin.tile([P, EC, S], bf16)
            for o in range(EC):
                nc.sync.dma_start(xk[:, o], xk_r[:, o])
            for o in range(EC):
                nc.sync.dma_start(xv[:, o], xv_r[:, o])
            for o in range(EC):
                nc.sync.dma_start(xq[:, o], xq_r[:, o])

            # ---- projections ----
            with tc.tile_pool(name="psA", bufs=2, space="PSUM") as psA:
                # K^T: psum [64, 512] = sum_o wk[:,o]^T @ xk[:,o,sblock]
                for sb in range(SB):
                    pk = psA.tile([HD, 512], f32, tag="pk")
                    for o in range(EC):
                        nc.tensor.matmul(
                            pk[:], wk[:, o], xk[:, o, 512 * sb:512 * (sb + 1)],
                            start=(o == 0), stop=(o == EC - 1),
                        )
                    nc.vector.tensor_copy(kt2[0:HD, 512 * sb:512 * (sb + 1)], pk[:])
                    nc.gpsimd.dma_start(kt2[HD:P, 512 * sb:512 * (sb + 1)],
                                      kt2[0:HD, 512 * sb:512 * (sb + 1)])

                # Q^T: psum [128, 512]; partitions 0:64 = head 2m, 64:128 = head 2m+1
                for m in range(2):
                    for sb in range(SB):
                        pq = psA.tile([P, 512], f32, tag="pq")
                        for o in range(EC):
                            nc.tensor.matmul(
                                pq[:], wq[:, o, P * m:P * (m + 1)],
                                xq[:, o, 512 * sb:512 * (sb + 1)],
                                start=(o == 0), stop=(o == EC - 1),
                            )
                        ss = slice(512 * sb, 512 * (sb + 1))
                        nc.vector.tensor_copy(qt2[0:HD, 2 * m, ss], pq[0:HD])
                        nc.vector.tensor_copy(qt2[HD:P, 2 * m + 1, ss], pq[HD:P])
                        nc.gpsimd.dma_start(qt2[HD:P, 2 * m, ss], qt2[0:HD, 2 * m, ss])
                        nc.gpsimd.dma_start(qt2[0:HD, 2 * m + 1, ss],
                                          qt2[HD:P, 2 * m + 1, ss])

                # V natural: psum [128, 64] = sum_o xv[:,o,schunk]^T @ wv[:,o]
                # (after K/Q so the scores+exp pipeline can start earlier)
                for sc in range(SC):
                    pv = psA.tile([P, HD], f32, tag="pv")
                    for o in range(EC):
                        nc.tensor.matmul(
                            pv[:], xv[:, o, P * sc:P * (sc + 1)], wv[:, o],
                            start=(o == 0), stop=(o == EC - 1),
                        )
                    nc.vector.tensor_copy(vaug[:, sc, 0:HD], pv[:])
                nc.vector.memset(vaug[:, :, HD], 1.0)

            # ---- attention ----
            with tc.tile_pool(name="psB", bufs=2, space="PSUM") as psB:
                for qb in range(SB):
                    for h in range(NHC):
                        ncj = 4 * qb + 4  # kj chunks needed (<= diagonal)
                        at = atpool.tile([P, NKJ, 512], bf16, tag="at")
                        op = psB.tile([P, 4, HD + 1], f32, tag="small")
                        for c0 in range(0, ncj, 3):
                            nb = min(3, ncj - c0)
                            st = psB.tile([P, 3, 512], f32, tag="st3")
                            for i in range(nb):
                                c = c0 + i
                                po = HD * (c % 2)
                                nc.tensor.matmul(
                                    st[:, i],
                                    kt2[po:po + HD, P * c:P * (c + 1)],
                                    qt2[po:po + HD, h, 512 * qb:512 * (qb + 1)],
                                    start=True, stop=True,
                                )
                            nc.scalar.activation(
                                at[:, c0:c0 + nb], st[:, 0:nb],
                                mybir.ActivationFunctionType.Exp,
                                scale=SCALE,
                            )
                            for i in range(nb):
                                c = c0 + i
                                j = c - 4 * qb
                                if j >= 0:
                                    w = P * (j + 1)
                                    nc.vector.tensor_mul(
                                        out=at[:, c, 0:w],
                                        in0=at[:, c, 0:w],
                                        in1=masks[:, j, 0:w],
                                    )
                        for sq in range(4):
                            for c in range(ncj):
                                nc.tensor.matmul(
                                    op[:, sq],
                                    at[:, c, P * sq:P * (sq + 1)],
                                    vaug[:, c],
                                    start=(c == 0), stop=(c == ncj - 1),
                                )
                        rc = rcpool.tile([P, 4], f32, tag="rc")
                        nc.vector.reciprocal(rc[:], op[:, :, HD])
                        nc.vector.tensor_mul(
                            out=outn[:, 4 * qb:4 * qb + 4, HD * h:HD * (h + 1)],
                            in0=op[:, :, 0:HD],
                            in1=rc[:, :, None].to_broadcast((P, 4, HD)),
                        )

            # ---- transpose attn out + final projection ----
            with tc.tile_pool(name="psC", bufs=3, space="PSUM") as psC:
                for m in range(2):
                    for sc in range(SC):
                        pt = psC.tile([P, P], bf16, tag="tr")
                        nc.tensor.transpose(
                            pt[:], outn[:, sc, P * m:P * (m + 1)], ident[:]
                        )
                        nc.vector.tensor_copy(outT[:, m, P * sc:P * (sc + 1)], pt[:])
                for sc in range(SC):
                    for eb in range(2):
                        py = psC.tile([P, 512], f32, tag="py")
                        for m in range(2):
                            nc.tensor.matmul(
                                py[:], outT[:, m, P * sc:P * (sc + 1)],
                                wo[:, m, 512 * eb:512 * (eb + 1)],
                                start=(m == 0), stop=(m == 1),
                            )
                        ys = rcpool.tile([P, 512], f32, tag="ystage")
                        nc.vector.tensor_copy(ys[:], py[:])
                        nc.sync.dma_start(y_r[:, sc, 512 * eb:512 * (eb + 1)], ys[:])

    nc.compile()
    return nc


def _get_nc():
    if "nc" not in _CACHE:
        _CACHE["nc"] = _build()
    return _CACHE["nc"]


def _prep_inputs(query, key, value, Wq, Wk, Wv, Wo):
    """Build the 8 per-core input maps (host-side shard + transpose + cast)."""
    in_maps = []
    for cid in range(8):
        b, g = cid // 4, cid % 4
        mlo, mhi = MPC * g, MPC * (g + 1)
        klo, khi = HD * g, HD * (g + 1)
        in_maps.append({
            "xqT": np.ascontiguousarray(query[b].T).astype(BF16),
            "xkT": np.ascontiguousarray(key[b].T).astype(BF16),
            "xvT": np.ascontiguousarray(value[b].T).astype(BF16),
            "wqT": np.ascontiguousarray(Wq[mlo:mhi].T).astype(BF16),
            "wkT": np.ascontiguousarray(Wk[klo:khi].T).astype(BF16),
            "wvT": np.ascontiguousarray(Wv[klo:khi].T).astype(BF16),
            "woT": np.ascontiguousarray(Wo[:, mlo:mhi].T).astype(BF16),
        })
    return in_maps


def kernel(query, key, value, attn_mask, Wq, Wk, Wv, Wo):
    from concourse.bass_utils import run_bass_kernel_spmd

    query = np.asarray(query, dtype=np.float32)
    key = np.asarray(key, dtype=np.float32)
    value = np.asarray(value, dtype=np.float32)
    Wq = np.asarray(Wq, dtype=np.float32)
    Wk = np.asarray(Wk, dtype=np.float32)
    Wv = np.asarray(Wv, dtype=np.float32)
    Wo = np.asarray(Wo, dtype=np.float32)

    nc = _get_nc()
    in_maps = _prep_inputs(query, key, value, Wq, Wk, Wv, Wo)
    res = run_bass_kernel_spmd(nc, in_maps, core_ids=list(range(8)))
    parts = np.stack([res.results[cid]["y"] for cid in range(8)])  # [8, S, E]
    parts = parts.reshape(B, NKV, S, E)
    out = parts.sum(axis=1, dtype=np.float64).astype(np.float32)
    return out



# revision 9
# speedup vs baseline: 336.6188x; 336.6188x over previous
"""GQA kernel for Trainium2 (Bass/Tile), 8 NeuronCores.

Problem: B=2, S=2048, E=1024, NH=16 q-heads, NKV=4 kv-heads (group size 4),
causal mask, fp32 in/out.

Sharding: core = (batch b, kv-group g); each of the 8 cores handles one
batch element and one KV group (4 q heads + their shared kv head). The
out-projection is column-sharded by group; host sums the 4 group partials
per batch.

v2 layout strategy (no on-device transposes at all):
  - host supplies x^T (E x S) so projections contract E on partitions
  - Q^T [64d, S] per head and K^T [64d, S] feed the scores matmul directly
  - scores are computed TRANSPOSED: S^T[kj, qi] psum [128, 512]; exp on ACT
    writes A^T chunks (bf16) directly (no max-subtraction: |scaled| <= ~4)
  - A@V uses V as the STATIONARY operand: vdup_c = [V_c | ones] (128 cols);
    one N=512 stream per (chunk, head) accumulates psum op[0:64] = O^T
    (unnormalized) and op[64:128] = softmax denominator replicated across
    64 partitions -- so normalization is one reciprocal_approx_fast + one
    tensor_mul on DVE, writing outT [m, s] for the final projection.
  - final projection consumes outT directly; psum DMAed straight to HBM.
  - K=64 score matmuls are packed 2-at-a-time onto PE row-groups 0-63 /
    64-127 (via base_partition), so the half-depth matmuls run concurrently.
  - schedule is software-pipelined: scores(h+1) overlaps AV(h) on PE while
    ACT exps; JIT Q/V projections + previous q-block's out-projection fill
    PE gaps to keep the HAM clock-gate warm.
"""

import numpy as np
import ml_dtypes

BF16 = ml_dtypes.bfloat16

B, S, E = 2, 2048, 1024
NH, NKV = 16, 4
HD = 64          # head dim
GS = NH // NKV   # 4 q heads per kv head
NHC = 4          # q heads per core
MPC = NHC * HD   # 256 q-out dims per core
P = 128
EC = E // P      # 8 contraction chunks for projections
SC = S // P      # 16 s-chunks of 128
SB = S // 512    # 4 s-blocks of 512
NKJ = S // P     # 16 kj chunks of 128
SCALE = 1.0 / 8.0  # 1/sqrt(HD)

_CACHE = {}


def _build():
    import concourse.bass as bass
    import concourse.tile as tile
    from concourse import bacc, mybir

    f32 = mybir.dt.float32
    bf16 = mybir.dt.bfloat16
    EXP = mybir.ActivationFunctionType.Exp

    nc = bacc.Bacc("TRN2", target_bir_lowering=False, debug=False)

    xq_d = nc.dram_tensor("xqT", [E, S], bf16, kind="ExternalInput").ap()
    xk_d = nc.dram_tensor("xkT", [E, S], bf16, kind="ExternalInput").ap()
    xv_d = nc.dram_tensor("xvT", [E, S], bf16, kind="ExternalInput").ap()
    wq_d = nc.dram_tensor("wqT", [E, MPC], bf16, kind="ExternalInput").ap()
    wk_d = nc.dram_tensor("wkT", [E, HD], bf16, kind="ExternalInput").ap()
    wv_d = nc.dram_tensor("wvT", [E, HD], bf16, kind="ExternalInput").ap()
    wo_d = nc.dram_tensor("woT", [MPC, E], bf16, kind="ExternalInput").ap()
    y_d = nc.dram_tensor("y", [S, E], f32, kind="ExternalOutput").ap()

    xq_r = xq_d.rearrange("(o p) s -> p o s", p=P)
    xk_r = xk_d.rearrange("(o p) s -> p o s", p=P)
    xv_r = xv_d.rearrange("(o p) s -> p o s", p=P)
    y_r = y_d.rearrange("(o p) e -> p o e", p=P)

    with tile.TileContext(nc) as tc:
        with (
            tc.tile_pool(name="const", bufs=1) as const,
            tc.tile_pool(name="xin", bufs=1) as xin,
            tc.tile_pool(name="at", bufs=2) as atpool,
            tc.tile_pool(name="rc", bufs=2) as rcpool,
            tc.tile_pool(name="ps", bufs=2, space="PSUM") as ps,
        ):
            # ---- persistent SBUF tensors
            wq = const.tile([P, EC, MPC], bf16)
            wk = const.tile([P, EC, HD], bf16)
            wv = const.tile([P, EC, HD], bf16)
            wo = const.tile([P, 2, E], bf16)
            masks = const.tile([P, 4, 512], bf16)
            qt2 = const.tile([P, NHC, S], bf16)   # [0:64]=Q^T_h, [64:128]=copy
            kt2 = const.tile([P, S], bf16)        # [0:64]=K^T,  [64:128]=copy
            vdup = const.tile([P, SC, P], bf16)   # [V_c | ones] per s-chunk
            outT = const.tile([P, 2, S], bf16)    # attn out^T [m, s], normalized

            nc.sync.dma_start(wq[:], wq_d.rearrange("(o p) m -> p o m", p=P))
            nc.sync.dma_start(wk[:], wk_d.rearrange("(o p) m -> p o m", p=P))
            nc.sync.dma_start(wv[:], wv_d.rearrange("(o p) m -> p o m", p=P))
            nc.sync.dma_start(wo[:], wo_d.rearrange("(o p) e -> p o e", p=P))

            # masks[j][p, fi] = 1.0 if fi - p - 128j >= 0 else 0.0
            nc.gpsimd.memset(masks[:], 1.0)
            for j in range(4):
                nc.gpsimd.affine_select(
                    out=masks[:, j],
                    in_=masks[:, j],
                    compare_op=mybir.AluOpType.is_ge,
                    fill=0.0,
                    base=-128 * j,
                    pattern=[[1, 512]],
                    channel_multiplier=-1,
                )

            # whole tile to 1.0 (contiguous memset); vproj overwrites cols
            # 0:HD per chunk, leaving the ones-half at HD:P intact.
            nc.vector.memset(vdup[:], 1.0)

            xq = xin.tile([P, EC, S], bf16)
            xk = xin.tile([P, EC, S], bf16)
            xv = xin.tile([P, EC, S], bf16)
            # xk first (K proj is first on PE), then per-sblock v/q so early
            # attention blocks can start before the full inputs land.
            for o in range(EC):
                nc.sync.dma_start(xk[:, o], xk_r[:, o])
            for sb in range(SB):
                ss = slice(512 * sb, 512 * (sb + 1))
                for o in range(EC):
                    nc.sync.dma_start(xv[:, o, ss], xv_r[:, o, ss])
                for o in range(EC):
                    nc.sync.dma_start(xq[:, o, ss], xq_r[:, o, ss])

            # ---- projection emitters ----
            def kproj():
                for sb in range(SB):
                    ss = slice(512 * sb, 512 * (sb + 1))
                    pk = ps.tile([P, 512], f32, tag="op")
                    for o in range(EC):
                        nc.tensor.matmul(
                            pk[0:HD], wk[:, o], xk[:, o, ss],
                            start=(o == 0), stop=(o == EC - 1),
                        )
                    nc.vector.tensor_copy(kt2[0:HD, ss], pk[0:HD])
                    nc.gpsimd.dma_start(kt2[HD:P, ss], kt2[0:HD, ss])

            def qproj(sb):
                ss = slice(512 * sb, 512 * (sb + 1))
                for m in range(2):
                    pq = ps.tile([P, 512], f32, tag="op")
                    for o in range(EC):
                        nc.tensor.matmul(
                            pq[:], wq[:, o, P * m:P * (m + 1)], xq[:, o, ss],
                            start=(o == 0), stop=(o == EC - 1),
                        )
                    nc.vector.tensor_copy(qt2[0:HD, 2 * m, ss], pq[0:HD])
                    nc.vector.tensor_copy(qt2[HD:P, 2 * m + 1, ss], pq[HD:P])
                    nc.gpsimd.dma_start(qt2[HD:P, 2 * m, ss], qt2[0:HD, 2 * m, ss])
                    nc.gpsimd.dma_start(qt2[0:HD, 2 * m + 1, ss],
                                        qt2[HD:P, 2 * m + 1, ss])

            def vproj(sc):
                pv = ps.tile([P, 512], f32, tag="op")
                for o in range(EC):
                    nc.tensor.matmul(
                        pv[:, 0:HD], xv[:, o, P * sc:P * (sc + 1)], wv[:, o],
                        start=(o == 0), stop=(o == EC - 1),
                    )
                nc.vector.tensor_copy(vdup[:, sc, 0:HD], pv[:, 0:HD])

            # ---- attention emitters ----
            def scores(h, qb, at):
                ncj = 4 * qb + 4
                qs = slice(512 * qb, 512 * (qb + 1))
                for c0 in range(0, ncj, 3):
                    nb = min(3, ncj - c0)
                    st = ps.tile([P, 3, 512], f32, tag="st3")
                    for i in range(nb):
                        c = c0 + i
                        po = HD * (c % 2)
                        nc.tensor.matmul(
                            st[:, i],
                            kt2[po:po + HD, P * c:P * (c + 1)],
                            qt2[po:po + HD, h, qs],
                            start=True, stop=True,
                        )
                    nc.scalar.activation(at[:, c0:c0 + nb], st[:, 0:nb], EXP,
                                         scale=SCALE)
                    for i in range(nb):
                        c = c0 + i
                        j = c - 4 * qb
                        if j >= 0:
                            w = P * (j + 1)
                            nc.vector.tensor_mul(
                                out=at[:, c, 0:w],
                                in0=at[:, c, 0:w],
                                in1=masks[:, j, 0:w],
                            )

            def av(h, qb, at):
                ncj = 4 * qb + 4
                op = ps.tile([P, 512], f32, tag="op")
                for c in range(ncj):
                    nc.tensor.matmul(
                        op[:], vdup[:, c], at[:, c],
                        start=(c == 0), stop=(c == ncj - 1),
                    )
                # normalization: partition shifts ONLY in copies; 2-in DVE ops
                # all read/write the same base partition (HW-proven shapes).
                mh = HD * (h % 2)
                rc = rcpool.tile([P, 512], f32, tag="rc")
                rcr = rcpool.tile([P, 512], f32, tag="rcr")
                nc.vector.tensor_copy(rc[0:HD], op[HD:P])
                nc.vector.reciprocal(rcr[0:HD], rc[0:HD])
                oslice = outT[mh:mh + HD, h // 2, 512 * qb:512 * (qb + 1)]
                if mh == 0:
                    nc.vector.tensor_mul(out=oslice, in0=op[0:HD], in1=rcr[0:HD])
                else:
                    nt = rcpool.tile([P, 512], bf16, tag="nt")
                    nc.vector.tensor_mul(out=nt[0:HD], in0=op[0:HD], in1=rcr[0:HD])
                    nc.vector.tensor_copy(oslice, nt[0:HD])

            def outproj(qb):
                for sc in range(4 * qb, 4 * qb + 4):
                    for eb in range(2):
                        es = slice(512 * eb, 512 * (eb + 1))
                        py = ps.tile([P, 512], f32, tag="op")
                        for m in range(2):
                            nc.tensor.matmul(
                                py[:], outT[:, m, P * sc:P * (sc + 1)],
                                wo[:, m, es],
                                start=(m == 0), stop=(m == 1),
                            )
                        ys = rcpool.tile([P, 512], f32, tag="ys")
                        nc.vector.tensor_copy(ys[:], py[:])
                        nc.sync.dma_start(y_r[:, sc, es], ys[:])

            # ---- schedule ----
            kproj()
            for sb in range(SB):
                qproj(sb)
            for sc in range(SC):
                vproj(sc)
            for qb in range(SB):
                prev = None
                for h in range(NHC):
                    at = atpool.tile([P, NKJ, 512], bf16, tag="at")
                    scores(h, qb, at)
                    if h == 0 and qb > 0:
                        outproj(qb - 1)
                    if prev is not None:
                        av(*prev)
                    prev = (h, qb, at)
                av(*prev)
            outproj(SB - 1)

    nc.compile()
    return nc


def _get_nc():
    if "nc" not in _CACHE:
        _CACHE["nc"] = _build()
    return _CACHE["nc"]


def _prep_inputs(query, key, value, Wq, Wk, Wv, Wo):
    """Build the 8 per-core input maps (host-side shard + transpose + cast)."""
    in_maps = []
    for cid in range(8):
        b, g = cid // 4, cid % 4
        mlo, mhi = MPC * g, MPC * (g + 1)
        klo, khi = HD * g, HD * (g + 1)
        in_maps.append({
            "xqT": np.ascontiguousarray(query[b].T).astype(BF16),
            "xkT": np.ascontiguousarray(key[b].T).astype(BF16),
            "xvT": np.ascontiguousarray(value[b].T).astype(BF16),
            "wqT": np.ascontiguousarray(Wq[mlo:mhi].T).astype(BF16),
            "wkT": np.ascontiguousarray(Wk[klo:khi].T).astype(BF16),
            "wvT": np.ascontiguousarray(Wv[klo:khi].T).astype(BF16),
            "woT": np.ascontiguousarray(Wo[:, mlo:mhi].T).astype(BF16),
        })
    return in_maps


def kernel(query, key, value, attn_mask, Wq, Wk, Wv, Wo):
    from concourse.bass_utils import run_bass_kernel_spmd

    query = np.asarray(query, dtype=np.float32)
    key = np.asarray(key, dtype=np.float32)
    value = np.asarray(value, dtype=np.float32)
    Wq = np.asarray(Wq, dtype=np.float32)
    Wk = np.asarray(Wk, dtype=np.float32)
    Wv = np.asarray(Wv, dtype=np.float32)
    Wo = np.asarray(Wo, dtype=np.float32)

    nc = _get_nc()
    in_maps = _prep_inputs(query, key, value, Wq, Wk, Wv, Wo)
    res = run_bass_kernel_spmd(nc, in_maps, core_ids=list(range(8)))
    parts = np.stack([res.results[cid]["y"] for cid in range(8)])  # [8, S, E]
    parts = parts.reshape(B, NKV, S, E)
    out = parts.sum(axis=1, dtype=np.float64).astype(np.float32)
    return out
